# revision 7
# baseline (speedup 1.0000x reference)
"""Trainium2 Bass kernel for nn_DependencyParsing (embedding_lookup).

Strategy (pure data-parallel over 8 NeuronCores, B=65536 -> 8192/core):
  - word_table cast to bf16, rows padded to 256B; word embeddings gathered
    feature-major straight into SBUF via SWDGE transpose dma_gather
    (512 idx / instruction HW cap), cycled over SWDGE queues.
  - pos/dep lookups use no gather: pe@Wp + de@Wd + combined bias is
    onehot @ proj. The 7x(50+45) = 665 classes + 1 bias row are packed
    densely into 6 class-tiles of 128; proj[g] = 4*table_s[c] @ W_s is
    precomputed on the HOST, quantized to fp8e4, and laid out as
    DoubleRow slabs [128, 2, 704] so each PAIR of class-tiles contracts
    in ONE DoubleRow matmul (3 DR matmuls per M-tile instead of 7 bf16).
    One-hots are DVE is_equal(u8 idx row vs f32 iota) -> fp8 pair tiles.
  - h' = 4h (x4 folded into Ww/proj on host so h'^3 sits in e4m3 range);
    h'3 = Square(h')*h' written as fp8 pair tiles; logits = h'3 @ (8*Wo)
    as 3 DoubleRow matmuls; softmax stays class-major: ACT Exp(psum/512
    + bo), ones-vector sum matmul, DVE reciprocal, PE broadcast, DVE
    multiply. Output lands [93, B_core] f32 and the host transposes.
  - Previous chunk's logits/sum/broadcast matmuls are interleaved between
    the current chunk's M-tiles so the PE never idles long enough for the
    HAM clock gate to re-throttle.
"""

import os

import numpy as np
import ml_dtypes

import concourse.bacc as bacc
import concourse.mybir as mybir
import concourse.tile as tile
from concourse.tile import add_dep_helper
from concourse.bass_utils import run_bass_kernel_spmd

B, T, D, H, V, NPOS, NDEP, OUT = 65536, 7, 100, 700, 32000, 50, 45, 93
NCORES = 8
B_CORE = B // NCORES
CHUNK = 512
P = 128
NT = 6            # class-tiles for the packed one-hot (665 classes + bias)
MT = [(0, 128), (128, 128), (256, 128), (384, 128), (512, 128), (640, 64)]
HP = 704          # per-token padded width of Ww
dt = mybir.dt
bf16 = ml_dtypes.bfloat16
f8e4 = ml_dtypes.float8_e4m3
NQ = int(os.environ.get("KERNEL_NQ", "2"))

_NC_CACHE = {}


def build_nc(b_core):
    n_chunks = b_core // CHUNK
    nc = bacc.Bacc(None, target_bir_lowering=False, num_swdge_queues=max(NQ, 2))
    with tile.TileContext(nc) as tc:
        with tc.tile_pool(name="dram", bufs=1, space="DRAM") as dram:
            word_tab = dram.tile([V + 1, 128], dt.bfloat16, kind="ExternalInput",
                                 name="word_tab", uniquify=False)
            widx_d = dram.tile([P, T * n_chunks * 32], dt.int16, kind="ExternalInput",
                               name="widx", uniquify=False)
            vidx_d = dram.tile([P, n_chunks * NT * CHUNK], dt.uint8,
                               kind="ExternalInput", name="vidx", uniquify=False)
            iota_d = dram.tile([P, NT], dt.float32, kind="ExternalInput",
                               name="iota6", uniquify=False)
            ww_d = dram.tile([P, T * HP], dt.bfloat16, kind="ExternalInput",
                             name="w_word", uniquify=False)
            proj_d = dram.tile([P, 3 * 2 * 704], dt.float8e4, kind="ExternalInput",
                               name="proj8", uniquify=False)
            wo_d = dram.tile([P, 3 * 2 * 96], dt.float8e4, kind="ExternalInput",
                             name="wo8", uniquify=False)
            bo_d = dram.tile([P, 1], dt.float32, kind="ExternalInput",
                             name="bo_pad", uniquify=False)
            out_d = dram.tile([OUT, b_core], dt.float32, kind="ExternalOutput",
                              name="out", uniquify=False)

            with (
                tc.tile_pool(name="const", bufs=1) as const,
                tc.tile_pool(name="wg", bufs=3) as wg_pool,
                tc.tile_pool(name="vx", bufs=3) as vx_pool,
                tc.tile_pool(name="oh", bufs=3) as oh_pool,
                tc.tile_pool(name="sq", bufs=3) as sq_pool,
                tc.tile_pool(name="h3", bufs=2) as h3_pool,
                tc.tile_pool(name="exq", bufs=2) as ex_pool,
                tc.tile_pool(name="rcq", bufs=2) as rc_pool,
                tc.tile_pool(name="opq", bufs=2) as op_pool,
                tc.tile_pool(name="hps", bufs=1, space="PSUM") as hps_pool,
                tc.tile_pool(name="ltps", bufs=2, space="PSUM") as ltps_pool,
            ):
                preloads = []
                ww_sb = const.tile([P, T * HP], dt.bfloat16, name="ww_sb")
                preloads.append(nc.sync.dma_start(out=ww_sb[:], in_=ww_d[:]))
                proj_sb = const.tile([P, 3, 2, 704], dt.float8e4, name="proj_sb")
                preloads.append(nc.sync.dma_start(
                    out=proj_sb.rearrange("p a b c -> p (a b c)"), in_=proj_d[:]))
                wo_sb = const.tile([P, 3, 2, 96], dt.float8e4, name="wo_sb")
                preloads.append(nc.sync.dma_start(
                    out=wo_sb.rearrange("p a b c -> p (a b c)"), in_=wo_d[:]))
                widx_sb = const.tile([P, T * n_chunks * 32], dt.int16, name="widx_sb")
                preloads.append(nc.sync.dma_start(out=widx_sb[:], in_=widx_d[:]))
                iota_sb = const.tile([P, NT], dt.float32, name="iota_sb")
                preloads.append(nc.sync.dma_start(out=iota_sb[:], in_=iota_d[:]))
                bo_sb = const.tile([P, 1], dt.float32, name="bo_sb")
                preloads.append(nc.sync.dma_start(out=bo_sb[:], in_=bo_d[:]))
                ones_col = const.tile([P, 1], dt.float32, name="ones_col")
                nc.vector.memset(ones_col[:, :], 1.0)
                ones_row = const.tile([1, 96], dt.float32, name="ones_row")
                nc.vector.memset(ones_row[:, :], 1.0)

                # Deferred epilogue pieces for the previous chunk.
                pend = {}

                def emit_logits(h3_list):
                    lg_ps = ltps_pool.tile([P, CHUNK], dt.float32, name="lg_ps",
                                           tag="lt")
                    for pn in range(3):
                        nc.tensor.matmul(
                            lg_ps[:96, :], wo_sb[:, pn, :, :], h3_list[pn][:, :, :],
                            start=(pn == 0), stop=(pn == 2),
                            perf_mode=mybir.MatmulPerfMode.DoubleRow)
                    ex = ex_pool.tile([P, CHUNK], dt.float32, name="ex")
                    # logits_ps = (4h)^3 @ (8 Wo) = 512 * (h3 @ Wo)
                    nc.scalar.activation(ex[:96, :], lg_ps[:96, :],
                                         mybir.ActivationFunctionType.Exp,
                                         bias=bo_sb[:96, :], scale=1.0 / 512.0)
                    pend["ex"] = ex

                def emit_sum():
                    sum_ps = ltps_pool.tile([P, CHUNK], dt.float32, name="sum_ps",
                                            tag="lt")
                    nc.tensor.matmul(sum_ps[:1, :], ones_col[:OUT, :],
                                     pend["ex"][:OUT, :], start=True, stop=True)
                    pend["sum_ps"] = sum_ps

                def emit_recip():
                    rc = rc_pool.tile([1, CHUNK], dt.float32, name="rc")
                    nc.vector.reciprocal(rc[:1, :], pend["sum_ps"][:1, :])
                    pend["rc"] = rc

                def emit_bcast(cc):
                    rcb_ps = ltps_pool.tile([P, CHUNK], dt.float32, name="rcb_ps",
                                            tag="lt")
                    nc.tensor.matmul(rcb_ps[:96, :], ones_row[:1, :96],
                                     pend["rc"][:1, :], start=True, stop=True)
                    opt = op_pool.tile([P, CHUNK], dt.float32, name="opt")
                    nc.vector.tensor_mul(opt[:OUT, :], pend["ex"][:OUT, :],
                                         rcb_ps[:OUT, :])
                    nc.sync.dma_start(out=out_d[:, cc * CHUNK:(cc + 1) * CHUNK],
                                      in_=opt[:OUT, :])

                qn = 0
                prev_h3 = None
                for c in range(n_chunks):
                    # ---- word gathers (feature-major), cycled over queues ----
                    wg = []
                    for t in range(T):
                        g = wg_pool.tile([P, CHUNK], dt.bfloat16, name=f"wg{t}")
                        gi = nc.gpsimd.dma_gather(
                            g.rearrange("p (o n) -> p o n", o=1),
                            word_tab[:],
                            widx_sb[:, (t * n_chunks + c) * 32:
                                    (t * n_chunks + c + 1) * 32],
                            CHUNK, CHUNK, 128, transpose=True, queue_num=qn % NQ,
                        )
                        if c == 0:
                            # keep transpose-gather traffic strictly after the
                            # preload DMAs (concurrent HWDGE transfers have
                            # been observed to corrupt gather/preload packets)
                            for pl in preloads:
                                add_dep_helper(gi.ins, pl.ins)
                        qn += 1
                        wg.append(g)

                    # ---- one-hot fp8 pair tiles: one fused is_equal ----
                    # (host pre-folds the class offset: vx == 128 <=> match)
                    vx = vx_pool.tile([P, NT, CHUNK], dt.uint8, name="vx")
                    nc.sync.dma_start(
                        out=vx.rearrange("p a b -> p (a b)"),
                        in_=vidx_d[:, c * NT * CHUNK:(c + 1) * NT * CHUNK])
                    oh_all = oh_pool.tile([P, 3, 2, CHUNK], dt.float8e4,
                                          name="oh_all")
                    nc.vector.tensor_scalar(
                        oh_all.rearrange("p a b c -> p (a b c)"),
                        vx.rearrange("p a b -> p (a b)"),
                        128.0, None, mybir.AluOpType.is_equal)
                    oh = [oh_all[:, pn, :, :] for pn in range(3)]

                    # ---- h' = x @ W' (bf16 word + DR one-hot), h'3 tiles ----
                    h3 = []
                    for mi, (m0, msz) in enumerate(MT):
                        hp = hps_pool.tile([P, CHUNK], dt.float32, name=f"hps{mi}")
                        for t in range(T):
                            nc.tensor.matmul(
                                hp[:msz, :],
                                ww_sb[:, t * HP + m0: t * HP + m0 + msz],
                                wg[t][:, :],
                                start=(t == 0), stop=False)
                        for pn in range(3):
                            nc.tensor.matmul(
                                hp[:msz, :], proj_sb[:, pn, :, m0:m0 + msz],
                                oh[pn], start=False, stop=(pn == 2),
                                perf_mode=mybir.MatmulPerfMode.DoubleRow)
                        sq = sq_pool.tile([P, CHUNK], dt.float32, name="sq")
                        nc.scalar.square(sq[:msz, :], hp[:msz, :])
                        if mi % 2 == 0:
                            h3p = h3_pool.tile([P, 2, CHUNK], dt.float8e4,
                                               name=f"h3p{mi // 2}")
                            h3.append(h3p)
                        else:
                            h3p = h3[mi // 2]
                        if mi == 5:
                            nc.vector.memset(h3p[64:128, 1, :], 0.0)
                        nc.vector.tensor_mul(h3p[:msz, mi % 2, :], sq[:msz, :],
                                             hp[:msz, :])
                        if prev_h3 is not None:
                            if mi == 0:
                                emit_logits(prev_h3)
                            elif mi == 2:
                                emit_sum()
                            elif mi == 3:
                                emit_recip()
                            elif mi == 4:
                                emit_bcast(c - 1)
                    prev_h3 = h3

                # tail epilogue for the last chunk
                emit_logits(prev_h3)
                emit_sum()
                emit_recip()
                emit_bcast(n_chunks - 1)
    nc.compile()
    return nc


def _wrap_idx(idx_tc):
    """[CHUNK] -> [128, 32] wrapped (i -> [i%16, i//16]) + replicated x8."""
    n = idx_tc.shape[0]
    w = idx_tc.reshape(n // 16, 16).T  # [16, n/16]
    return np.tile(w, (8, 1))


def prep_inputs(word_idx, pos_idx, dep_idx, word_table, pos_table, dep_table,
                Ww, bw, Wp, bp, Wd, bd, Wo, bo, b_core):
    """Returns (shared_map, per_core_fn). Host work is layout + small matmuls."""
    n_chunks = b_core // CHUNK

    wt = np.zeros((V + 1, 128), dtype=bf16)
    wt[:V, :D] = np.asarray(word_table, np.float32).astype(bf16)

    # h' = 4h -> scale Ww, proj, biases by 4
    def pack_w(Wmat):
        arr = np.zeros((P, T, HP), dtype=bf16)
        Wmat = np.asarray(Wmat, np.float32) * 4.0
        for t in range(T):
            arr[:D, t, :H] = Wmat[D * t:D * (t + 1), :].astype(bf16)
        return arr.reshape(P, T * HP)

    ww = pack_w(Ww)

    # host proj: [666, 700] f32 -> fp8 DR slabs [128, 3, 2, 704]
    pt = np.asarray(pos_table, np.float32)
    dtab = np.asarray(dep_table, np.float32)
    Wp32 = np.asarray(Wp, np.float32)
    Wd32 = np.asarray(Wd, np.float32)
    proj = np.zeros((3 * 2 * P, H), np.float32)
    for t in range(T):
        proj[95 * t:95 * t + 50] = 4.0 * (pt @ Wp32[D * t:D * (t + 1)])
        proj[95 * t + 50:95 * t + 95] = 4.0 * (dtab @ Wd32[D * t:D * (t + 1)])
    proj[665] = 4.0 * (np.asarray(bw, np.float32) + np.asarray(bp, np.float32)
                       + np.asarray(bd, np.float32))
    proj8 = np.zeros((P, 3, 2, 704), dtype=f8e4)
    for pn in range(3):
        for j in range(2):
            rows = proj[128 * (2 * pn + j):128 * (2 * pn + j + 1)]  # [128, 700]
            proj8[:, pn, j, :H] = rows.astype(f8e4)

    # Wo * 8 as DR slabs [128, 3, 2, 96]
    Wo32 = np.asarray(Wo, np.float32) * 8.0
    wo8 = np.zeros((P, 3, 2, 96), dtype=f8e4)
    for pn in range(3):
        for j in range(2):
            k0 = 128 * (2 * pn + j)
            ksz = min(128, H - k0)
            if ksz > 0:
                wo8[:ksz, pn, j, :OUT] = Wo32[k0:k0 + ksz].astype(f8e4)

    bo_pad = np.zeros((P, 1), dtype=np.float32)
    bo_pad[:OUT, 0] = np.asarray(bo, np.float32)

    # class offsets folded into vx on host: vx' = val - c + 128; match <=> 128
    iota6 = np.zeros((P, NT), np.float32)  # unused on device now
    cmap = np.full((NT, P), 255, np.int64)
    smap = np.zeros((NT, P), np.int64)  # stream id; 14 = bias/dead
    for tau in range(NT):
        for p in range(P):
            g = 128 * tau + p
            if g < 665:
                tb, r = divmod(g, 95)
                if r < 50:
                    smap[tau, p] = 2 * tb
                    cmap[tau, p] = r
                else:
                    smap[tau, p] = 2 * tb + 1
                    cmap[tau, p] = r - 50
            elif g == 665:
                smap[tau, p] = 14
                cmap[tau, p] = 0
            else:
                smap[tau, p] = 14  # val 0, c 255 -> 129 != 128 never matches

    shared = {
        "word_tab": wt,
        "w_word": ww,
        "proj8": np.ascontiguousarray(proj8).reshape(P, 3 * 2 * 704),
        "wo8": np.ascontiguousarray(wo8).reshape(P, 3 * 2 * 96),
        "bo_pad": bo_pad,
        "iota6": iota6,
    }

    wi = np.asarray(word_idx, np.int64).copy()
    wi[wi < 0] = V
    wi = wi.astype(np.int16)
    pi8 = np.asarray(pos_idx, np.int32).astype(np.uint8)
    di8 = np.asarray(dep_idx, np.int32).astype(np.uint8)

    def core_map(core):
        s = slice(core * b_core, (core + 1) * b_core)
        wic = wi[s]
        widx = np.zeros((P, T, n_chunks, 32), dtype=np.int16)
        for t in range(T):
            for c in range(n_chunks):
                widx[:, t, c, :] = _wrap_idx(wic[c * CHUNK:(c + 1) * CHUNK, t])

        # idx streams [15, b_core]: 2t = pos_t, 2t+1 = dep_t, 14 = zeros
        streams = np.zeros((15, b_core), np.uint8)
        for t in range(T):
            streams[2 * t] = pi8[s, t]
            streams[2 * t + 1] = di8[s, t]
        vxf = (streams[smap].astype(np.int16) - cmap[:, :, None] + 128) % 256
        vxf = vxf.astype(np.uint8).reshape(NT, P, n_chunks, CHUNK)
        vx = np.ascontiguousarray(vxf.transpose(1, 2, 0, 3))

        m = dict(shared)
        m["widx"] = widx.reshape(P, T * n_chunks * 32)
        m["vidx"] = vx.reshape(P, n_chunks * NT * CHUNK)
        return m

    return shared, core_map


def kernel(**inputs):
    b_core = B_CORE
    if b_core not in _NC_CACHE:
        _NC_CACHE[b_core] = build_nc(b_core)
    nc = _NC_CACHE[b_core]

    _, core_map = prep_inputs(b_core=b_core, **inputs)
    in_maps = [core_map(i) for i in range(NCORES)]
    res = run_bass_kernel_spmd(nc, in_maps, core_ids=list(range(NCORES)))
    out = np.concatenate([r["out"] for r in res.results], axis=1)  # [93, B]
    return np.ascontiguousarray(out.T).astype(np.float32)
